# revision 14
# baseline (speedup 1.0000x reference)
"""LoRA linear kernel for 8 Trainium2 NeuronCores.

Computes out = x @ W.T + b + 2.0 * (x @ (A @ B.T).T) for
x:[2,4096,4096] W:[4096,4096] b:[4096] A:[4096,8] B:[4096,8] (all f32).

Strategy: dp=2 (batch/seq rows) x tp=4 (out features) grid over 8 cores.
Per core: cache W^T shard [4096,1024] in SBUF, fold the rank-8 LoRA update
(2 * B @ A_shard^T) into the cached W^T on-device with K=8 matmuls, then a
single streamed GEMM out = x_shard @ W_eff^T with the bias added via a K=1
ones-outer-product matmul into the same PSUM accumulation group. Matmuls run
as float32r (TF32-like) which is full PE rate for moving dim >= 256.

Host side only reshapes/transposes/slices the inputs (layout prep for DMA
efficiency); all arithmetic happens on device.
"""

import sys

sys.path.insert(0, "/opt/trn_rl_repo")

import numpy as np

P = 128
B_, S, DIN, DOUT = 2, 4096, 4096, 4096
R = 8
DP, TP = 2, 4
M = B_ * S          # 8192 total rows
M_C = M // DP       # 4096 rows per core
N_C = DOUT // TP    # 1024 out features per core
KT = DIN // P       # 32 k-tiles
NCHUNK = 512
NCH = N_C // NCHUNK  # 2 n-chunks
MT = M_C // P       # 32 m-tiles

_compiled = {}


def _build():
    import concourse.tile as tile
    from concourse import bacc, mybir

    f32 = mybir.dt.float32
    f32r = mybir.dt.float32r

    nc = bacc.Bacc("TRN2", target_bir_lowering=False, debug=False, num_devices=DP * TP)

    xT = nc.dram_tensor("xT", [DIN, M_C], f32, kind="ExternalInput").ap()
    Wt = nc.dram_tensor("Wt", [DIN, N_C], f32, kind="ExternalInput").ap()
    Bt = nc.dram_tensor("Bt", [R, DIN], f32, kind="ExternalInput").ap()
    At = nc.dram_tensor("At", [R, N_C], f32, kind="ExternalInput").ap()
    bias = nc.dram_tensor("bias", [1, N_C], f32, kind="ExternalInput").ap()
    out = nc.dram_tensor("out", [M_C, N_C], f32, kind="ExternalOutput").ap()

    with tile.TileContext(nc) as tc:
        with (
            tc.tile_pool(name="wt", bufs=1) as wt_pool,
            tc.tile_pool(name="const", bufs=1) as const_pool,
            tc.tile_pool(name="x", bufs=2) as x_pool,
            tc.tile_pool(name="pre_x", bufs=2) as pre_x_pool,
            tc.tile_pool(name="o", bufs=2) as o_pool,
            tc.tile_pool(name="psum", bufs=8, space="PSUM") as psum_pool,
        ):
            NPRE = 2  # m-tiles interleaved with the W^T preload / LoRA fold

            def x_panel(m):
                xm = x_pool.tile([P, KT * P], f32r, tag="xm")
                nc.gpsimd.dma_start(
                    xm[:].rearrange("p (k s) -> p k s", s=P),
                    xT[:, m * P : (m + 1) * P].bitcast(f32r).rearrange("(k p) s -> p k s", p=P),
                )
                return xm

            def evict(m, n, ps):
                om = o_pool.tile([P, NCHUNK], f32, tag="om")
                nc.vector.tensor_copy(om[:], ps[:])
                nc.sync.dma_start(
                    out[m * P : (m + 1) * P, n * NCHUNK : (n + 1) * NCHUNK], om[:]
                )

            # ---- small constants (HWDGE queue, ahead of W^T slices) ----
            bt_sb = const_pool.tile([R, DIN], f32r)
            nc.sync.dma_start(bt_sb[:], Bt[:].bitcast(f32r))
            at_sb = const_pool.tile([R, N_C], f32)
            nc.sync.dma_start(at_sb[:], At[:])
            at2 = const_pool.tile([R, N_C], f32r)
            nc.vector.tensor_scalar_mul(at2[:], at_sb[:], 2.0)
            bias_sb = const_pool.tile([1, N_C], f32r)
            nc.sync.dma_start(bias_sb[:], bias[:].bitcast(f32r))
            ones_sb = const_pool.tile([1, P], f32r)
            nc.vector.memset(ones_sb[:].bitcast(f32), 1.0)

            def bias_mm(ps, n):
                nc.tensor.matmul(
                    ps[:],
                    ones_sb[:],
                    bias_sb[:, n * NCHUNK : (n + 1) * NCHUNK],
                    start=False,
                    stop=True,
                )

            # ---- W^T preload + LoRA fold + first NPRE m-tiles, pipelined per k ----
            wt_sb = wt_pool.tile([P, KT * N_C], f32r)  # [p, k*N_C + o] = Wt[k*128+p, o]

            def wt_slice(k, n):
                return wt_sb[:, k * N_C + n * NCHUNK : k * N_C + (n * NCHUNK + NCHUNK)]

            pre_ps = [
                [
                    psum_pool.tile([P, NCHUNK], f32, tag="ps", name=f"ps_pre_{mi}_{n}")
                    for n in range(NCH)
                ]
                for mi in range(NPRE)
            ]
            panels = {}
            for k in range(KT):
                nc.sync.dma_start(
                    wt_sb[:, k * N_C : (k + 1) * N_C],
                    Wt[k * P : (k + 1) * P, :].bitcast(f32r),
                )
                # x^T slice [128 i, NPRE*128 s] for this k, first NPRE m-tiles
                px = pre_x_pool.tile([P, NPRE * P], f32r, tag="px", name=f"px_{k}")
                nc.sync.dma_start(
                    px[:], xT[k * P : (k + 1) * P, 0 : NPRE * P].bitcast(f32r)
                )
                for n in range(NCH):
                    psf = psum_pool.tile([P, NCHUNK], f32, tag="ps", name=f"psf_{k}_{n}")
                    nc.tensor.matmul(
                        psf[:],
                        bt_sb[:, k * P : (k + 1) * P],
                        at2[:, n * NCHUNK : (n + 1) * NCHUNK],
                        start=True,
                        stop=True,
                    )
                    sl = wt_slice(k, n)
                    nc.vector.tensor_add(sl, sl.bitcast(f32), psf[:])
                for mi in range(NPRE):
                    for n in range(NCH):
                        nc.tensor.matmul(
                            pre_ps[mi][n][:],
                            px[:, mi * P : (mi + 1) * P],
                            wt_slice(k, n),
                            start=(k == 0),
                            stop=False,
                        )
                # prefetch the first steady-state panels mid-preload
                if k in (20, 26):
                    mpre = NPRE + (0 if k == 20 else 1)
                    panels[mpre] = x_panel(mpre)
            for mi in range(NPRE):
                for n in range(NCH):
                    bias_mm(pre_ps[mi][n], n)
                    evict(mi, n, pre_ps[mi][n])

            # ---- steady-state m-tiles ----
            for m in range(NPRE, MT):
                xm = panels.pop(m, None)
                if xm is None:
                    xm = x_panel(m)
                for n in range(NCH):
                    ps = psum_pool.tile([P, NCHUNK], f32, tag="ps")
                    for k in range(KT):
                        nc.tensor.matmul(
                            ps[:],
                            xm[:, k * P : (k + 1) * P],
                            wt_slice(k, n),
                            start=(k == 0),
                            stop=False,
                        )
                    bias_mm(ps, n)
                    evict(m, n, ps)

    nc.compile()
    return nc


def _get_nc():
    if "nc" not in _compiled:
        _compiled["nc"] = _build()
    return _compiled["nc"]


def kernel(x: np.ndarray, W: np.ndarray, b: np.ndarray, A: np.ndarray, B: np.ndarray) -> np.ndarray:
    from concourse.bass_utils import run_bass_kernel_spmd

    x = np.ascontiguousarray(np.asarray(x, dtype=np.float32))
    W = np.asarray(W, dtype=np.float32)
    b = np.asarray(b, dtype=np.float32)
    A = np.asarray(A, dtype=np.float32)
    B = np.asarray(B, dtype=np.float32)

    nc = _get_nc()

    xf = x.reshape(M, DIN)
    Bt_host = np.ascontiguousarray(B.T)  # [R, DIN]

    in_maps = []
    for c in range(DP * TP):
        d, t = divmod(c, TP)
        in_maps.append(
            {
                "xT": np.ascontiguousarray(xf[d * M_C : (d + 1) * M_C, :].T),
                "Wt": np.ascontiguousarray(W[t * N_C : (t + 1) * N_C, :].T),
                "Bt": Bt_host,
                "At": np.ascontiguousarray(A[t * N_C : (t + 1) * N_C, :].T),
                "bias": np.ascontiguousarray(b[t * N_C : (t + 1) * N_C].reshape(1, N_C)),
            }
        )

    res = run_bass_kernel_spmd(nc, in_maps, list(range(DP * TP)))

    outf = np.empty((M, DOUT), dtype=np.float32)
    for c in range(DP * TP):
        d, t = divmod(c, TP)
        outf[d * M_C : (d + 1) * M_C, t * N_C : (t + 1) * N_C] = res.results[c]["out"]
    return outf.reshape(B_, S, DOUT)
